# revision 2
# baseline (speedup 1.0000x reference)
"""GCNConv Trainium2 kernel, 8-core SPMD.

Math: out = D^-1/2 A D^-1/2 (x W^T + b), A = adjacency (+self loops,
duplicate edges collapse to 1).

Reformulated aggregate-first so no cross-core communication is needed:
    s    = deg^-1/2                       (host, from dedup'd A)
    xs   = SCALE * s ⊙ x                  (host)
    xh   = fp8(xs); xl = fp8(xs - xh)     (host hi/lo split, e4m3)
    agg  = A @ (xh + xl)                  (device matmul 1, row-sharded,
                                           fp8 DoubleRow: 2 k-tiles/instr
                                           at 0.5 cyc/row = 2x fp16 rate)
    aggs = SCALE * A @ s                  (host matvec, feeds bias term)
    out  = (s/SCALE) ⊙ ([agg, aggs] @ [W^T; b])  (device matmul 2 + fused
                                           scale on PSUM->SBUF eviction)

Device per core c (rows r = c*1024 .. c*1024+1024):
  matmul 1: aggT[f, r] = sum_j (xh+xl)[j, f] * AT[j, r].  Operands both
            fp8e4m3 packed as [128p, 2 k-tiles, free] pairs; DoubleRow
            processes both k-tiles in one instruction.  A entries are
            0/1 -> exact in fp8; xs hi+lo recovers ~8 mantissa bits.
  matmul 2: out[r, o] = sum_f aggT[f, r] * Wt[f, o] + aggs[r] * b[o]
            then scaled by s[r]/SCALE on PSUM->SBUF eviction (ACT Copy).

Full inputs in, full outputs out; sharding is internal (each core gets its
own AT slice / aggs slice / s slice; xh, xl, Wt, b broadcast).
"""

import functools
import numpy as np

N = 8192
D = 512
NCORES = 8
ROWS = N // NCORES          # 1024 output rows per core
P = 128
KT = N // P                 # 64 contraction tiles
KP = KT // 2                # 32 DoubleRow k-tile pairs
FT = D // P                 # 4 feature tiles
NH = ROWS // D              # 2 row halves of 512 per core
MT = ROWS // P              # 8 output row chunks per core

SCALE = 32.0                # keeps the fp8 lo part in normal range


def _kernel_body(tc, aps, bufs=8):
    import concourse.mybir as mybir

    nc = tc.nc
    at, xh, xl, wt, brow, aggs, sc, out = (
        aps["at"], aps["xh"], aps["xl"], aps["wt"], aps["brow"],
        aps["aggs"], aps["sc"], aps["out"],
    )
    half = mybir.dt.float16
    fp8 = mybir.dt.float8e4
    f32 = mybir.dt.float32
    DR = mybir.MatmulPerfMode.DoubleRow

    with (
        tc.tile_pool(name="xh_pool", bufs=bufs) as xh_pool,
        tc.tile_pool(name="xl_pool", bufs=bufs) as xl_pool,
        tc.tile_pool(name="at_pool", bufs=bufs) as at_pool,
        tc.tile_pool(name="psum", bufs=1, space="PSUM") as psum_pool,
        tc.tile_pool(name="aggT_pool", bufs=NH * FT) as aggT_pool,
        tc.tile_pool(name="out_pool", bufs=3) as out_pool,
        tc.tile_pool(name="const", bufs=1) as const,
    ):
        wt_sb = []
        b_sb = aggs_sb = s_sb = None

        def emit_consts():
            nonlocal b_sb, aggs_sb, s_sb
            for i in range(FT):
                w_t = const.tile([P, D], half, tag="wt", bufs=FT,
                                 name=f"wt{i}")
                nc.sync.dma_start(out=w_t[:], in_=wt[i * P:(i + 1) * P, :])
                wt_sb.append(w_t)
            b_sb = const.tile([1, D], half, tag="b", name="b_sb")
            nc.sync.dma_start(out=b_sb[:], in_=brow[:])
            aggs_sb = const.tile([1, ROWS], half, tag="aggs", name="aggs_sb")
            nc.sync.dma_start(out=aggs_sb[:], in_=aggs[:])
            s_sb = const.tile([P, MT], f32, tag="s", name="s_sb")
            nc.sync.dma_start(out=s_sb[:], in_=sc[:])

        # ---- matmul 1: aggregation  aggT[f] += (xh|xl)[kp].T @ at[kp] ----
        # fp8 DoubleRow: lhsT [128, 2, 128f], rhs [128, 2, 1024r] -> one
        # instruction covers two 128-row k-tiles at 0.5 cycles per output
        # column (2x the fp16 rate).
        psum = []
        for f in range(FT):
            ps = psum_pool.tile([P, ROWS], f32, tag=f"ps{f}", name=f"ps{f}")
            psum.append(ps)
        for kp in range(KP):
            xh_t = xh_pool.tile([P, 2, D], fp8, tag="xh", name=f"xh{kp}")
            nc.sync.dma_start(out=xh_t[:], in_=xh[kp * P:(kp + 1) * P, :, :])
            xl_t = xl_pool.tile([P, 2, D], fp8, tag="xl", name=f"xl{kp}")
            nc.sync.dma_start(out=xl_t[:], in_=xl[kp * P:(kp + 1) * P, :, :])
            at_t = at_pool.tile([P, 2, ROWS], fp8, tag="at", name=f"at{kp}")
            nc.sync.dma_start(out=at_t[:], in_=at[kp * P:(kp + 1) * P, :, :])
            if kp == 3:
                emit_consts()
            for f in range(FT):
                nc.tensor.matmul(
                    psum[f][:], xh_t[:, :, f * P:(f + 1) * P], at_t[:, :, :],
                    start=(kp == 0), stop=False, perf_mode=DR,
                )
                nc.tensor.matmul(
                    psum[f][:], xl_t[:, :, f * P:(f + 1) * P], at_t[:, :, :],
                    start=False, stop=(kp == KP - 1), perf_mode=DR,
                )

        # evict (fp32 -> fp16 cast); aggT[n*FT+f] is [128f, 512r] of half n
        aggT = [None] * (NH * FT)
        for f in range(FT):
            for n in range(NH):
                agg_t = aggT_pool.tile([P, D], half, tag="aggT",
                                       name=f"aggT{n}_{f}")
                nc.vector.tensor_copy(agg_t[:], psum[f][:, n * D:(n + 1) * D])
                aggT[n * FT + f] = agg_t

        # ---- matmul 2 + fused s-scale on eviction ----
        for m in range(MT):
            n, off = m // FT, (m % FT) * P
            # reuse the aggregation psum banks (same tag -> same slots)
            ps2 = psum_pool.tile([P, D], f32, tag=f"ps{m % 2}",
                                 name=f"ps2_{m}")
            for kf in range(FT):
                nc.tensor.matmul(
                    ps2[:],
                    aggT[n * FT + kf][:, off:off + P],
                    wt_sb[kf][:],
                    start=(kf == 0),
                    stop=False,
                )
            nc.tensor.matmul(
                ps2[:],
                aggs_sb[:, m * P:(m + 1) * P],
                b_sb[:],
                start=False,
                stop=True,
            )
            o_t = out_pool.tile([P, D], f32, tag="o", name=f"o{m}")
            nc.scalar.activation(
                o_t[:], ps2[:], mybir.ActivationFunctionType.Copy,
                scale=s_sb[:, m:m + 1],
            )
            nc.sync.dma_start(out=out[m * P:(m + 1) * P, :], in_=o_t[:])


@functools.lru_cache(maxsize=8)
def _build(repeat=1, bufs=8):
    import concourse.bacc as bacc
    import concourse.mybir as mybir
    import concourse.tile as tile

    half = mybir.dt.float16
    fp8 = mybir.dt.float8e4
    nc = bacc.Bacc("TRN2", target_bir_lowering=False, debug=False,
                   num_devices=NCORES)
    aps = {
        "at": nc.dram_tensor("at", [KP * P, 2, ROWS], fp8,
                             kind="ExternalInput").ap(),
        "xh": nc.dram_tensor("xh", [KP * P, 2, D], fp8,
                             kind="ExternalInput").ap(),
        "xl": nc.dram_tensor("xl", [KP * P, 2, D], fp8,
                             kind="ExternalInput").ap(),
        "wt": nc.dram_tensor("wt", [D, D], half, kind="ExternalInput").ap(),
        "brow": nc.dram_tensor("brow", [1, D], half, kind="ExternalInput").ap(),
        "aggs": nc.dram_tensor("aggs", [1, ROWS], half,
                               kind="ExternalInput").ap(),
        "sc": nc.dram_tensor("sc", [P, MT], mybir.dt.float32,
                             kind="ExternalInput").ap(),
        "out": nc.dram_tensor("out", [ROWS, D], mybir.dt.float32,
                              kind="ExternalOutput").ap(),
    }
    with tile.TileContext(nc) as tc:
        for _ in range(repeat):
            _kernel_body(tc, aps, bufs=bufs)
    nc.compile()
    return nc


def _pack_pairs(arr):
    """[8192, C] -> [4096, 2, C]: row (2kp+i)*128+p -> [kp*128+p, i, :]."""
    C = arr.shape[1]
    return np.ascontiguousarray(
        arr.reshape(KP, 2, P, C).transpose(0, 2, 1, 3).reshape(KP * P, 2, C))


def _prep(x, edge_index, W, b):
    """Host-side index scatter + scaling; returns per-core input maps."""
    import ml_dtypes
    half = np.float16
    fp8 = ml_dtypes.float8_e4m3
    ei = np.asarray(edge_index)
    # AT[j, r] = A[r, j]; duplicates collapse via assignment, + self loops
    AT = np.zeros((N, N), dtype=np.uint8)
    AT[ei[1].astype(np.int64), ei[0].astype(np.int64)] = 1
    idx = np.arange(N)
    AT[idx, idx] = 1
    deg = AT.sum(axis=0, dtype=np.int64).astype(np.float64)  # A row sums
    s = (1.0 / np.sqrt(deg)).astype(np.float32)
    aggs = (SCALE * (AT.T.astype(np.float32) @ s)).astype(half)  # SCALE*A@s
    xs = (SCALE * s[:, None] * np.asarray(x)).astype(np.float32)
    xh8 = xs.astype(fp8)
    xl8 = (xs - xh8.astype(np.float32)).astype(fp8)
    xh_p = _pack_pairs(xh8)
    xl_p = _pack_pairs(xl8)
    wt = np.ascontiguousarray(np.asarray(W).T).astype(half)
    brow = np.asarray(b).reshape(1, D).astype(half)
    s_out = (s / SCALE).astype(np.float32)

    in_maps = []
    for c in range(NCORES):
        rows = slice(c * ROWS, (c + 1) * ROWS)
        in_maps.append({
            "at": _pack_pairs(AT[:, rows]).astype(fp8),
            "xh": xh_p,
            "xl": xl_p,
            "wt": wt,
            "brow": brow,
            "aggs": np.ascontiguousarray(aggs[rows]).reshape(1, ROWS),
            # sc[p, m] = (s/SCALE)[c*1024 + m*128 + p]
            "sc": np.ascontiguousarray(
                s_out[rows].reshape(MT, P).T).astype(np.float32),
        })
    return in_maps


def kernel(x, edge_index, W, b):
    import time
    from concourse import bass_utils

    nc = _build()
    in_maps = _prep(x, edge_index, W, b)
    last = None
    for attempt in range(3):
        try:
            res = bass_utils.run_bass_kernel_spmd(
                nc, in_maps, core_ids=list(range(NCORES)))
            return np.concatenate(
                [res.results[c]["out"] for c in range(NCORES)], axis=0)
        except Exception as e:  # transient NRT device flakes recover on retry
            last = e
            time.sleep(5.0)
    raise last
